# revision 20
# baseline (speedup 1.0000x reference)
"""Multi-head attention (B=2, N=2048, D=1024, H=16) on 8 NeuronCores.

Sharding: data-parallel over batch (cores 0-3 -> b=0, cores 4-7 -> b=1),
tensor-parallel over heads (4 heads per core; column-parallel QKV,
row-parallel proj). Each core emits a partial projection output
y_c = O_heads(c) @ proj_w[rows(c)]; the host sums the 4 partials per batch
and adds proj_b.

v2 design (ACT-exp is the hard floor: 16.8M exps/core @ 1.2GHz ~= 110us):
  - The Scalar (ACT) engine does ONLY exp, one [128,1024] activate per
    (block, mt).  All copies / bias adds run on DVE.
  - Stage B runs as 8 blocks (n-chunk of 512 x head-pair).  Per mt:
    two K=64 score matmuls row-tiled to PE quadrants (0,0)/(64,0) so they
    execute concurrently, one exp, two U (E^T[V|1]) matmuls with M=65
    (row 64 accumulates the softmax denominator).
  - PSUM: st 2x[128,1024] (4 banks) + us 2x[65,512] (2 banks) +
    aux 2x[128,512] (2 banks) = 8 banks exactly.  aux serves transposes,
    QKV chunk accumulation, and proj accumulation.
  - Residual stage-A work (later q/k chunks) and the projection are cut
    into <=1024-cycle steps and statically interleaved into the PE slack
    of stage-B mt slots; U lags ST by ULAG slots so the block-boundary
    normalize (DVE) never stalls the exp cadence.
  - Normalize: DVE copy of U rows, DVE reciprocal of the denominator row,
    DRAM-bounce broadcast, DVE multiply into OT (c-major), as in v1.
"""

import numpy as np

import concourse.bass as bass
import concourse.tile as tile
from concourse import mybir
from concourse.bass_utils import run_bass_kernel_spmd
from concourse.masks import make_identity

# ---- problem constants (hardcoded per contract) ----
B = 2
N = 2048
D = 1024
H = 16
HD = 64          # head dim
SCALE = HD ** -0.5
NC = 8           # cores
HL = H // (NC // B)   # heads per core = 4
CW = HL * HD     # local qkv column width = 256

F32 = mybir.dt.float32
F32R = mybir.dt.float32r

NT = N // 128    # 16 m-tiles
KC = D // 128    # 8 contraction chunks for qkv matmuls
NCK = N // 512   # 4 n-chunks (stage-B block columns)
ULAG = 2         # U matmuls lag ST by this many mt slots

ALU = mybir.AluOpType
AF = mybir.ActivationFunctionType


def _mm(ap):
    return ap.bitcast(F32R)


def _r(ap):
    return ap.bitcast(F32R)


def _split_sync_waits(nc, maxw: int = 1) -> int:
    """This walrus build rejects >1 semaphore-wait per instruction
    (setupSyncWait: "Too many sync wait commands"). Hoist excess waits
    onto preceding same-engine no-ops: the sequencer runs instructions
    in order, so the semantics are unchanged."""
    n_split = 0
    for fn in nc.m.functions:
        for bb in fn.blocks:
            insts = list(bb.instructions)
            out = []
            changed = False
            for inst in insts:
                si = inst.sync_info
                waits = list(si.on_wait) if si is not None and si.on_wait else []
                if len(waits) > maxw:
                    chunks = [waits[i: i + maxw] for i in range(0, len(waits), maxw)]
                    for chunk in chunks[:-1]:
                        out.append(mybir.InstNoOp(
                            name=f"I-splitw-{nc.next_id()}",
                            sync_info=mybir.SyncInfo(on_wait=chunk, on_update=[]),
                            bass_nofuse=True,
                            engine=inst.engine,
                        ))
                    si.on_wait = chunks[-1]
                    inst.sync_info = si
                    n_split += 1
                    changed = True
                out.append(inst)
            if changed:
                try:
                    bb.instructions = out
                except Exception:
                    bb.instructions.clear()
                    for i in out:
                        bb.instructions.append(i)
    return n_split


def _build_program(split=True, reps=1, stages="ABC"):
    nc = bass.Bass(trn_type="TRN2", target_bir_lowering=False, debug=False)

    x_d = nc.dram_tensor("x", [N, D], F32, kind="ExternalInput").ap()
    wq_d = nc.dram_tensor("wq", [D, CW], F32, kind="ExternalInput").ap()
    wk_d = nc.dram_tensor("wk", [D, CW], F32, kind="ExternalInput").ap()
    wv_d = nc.dram_tensor("wv", [D, CW], F32, kind="ExternalInput").ap()
    wp_d = nc.dram_tensor("wp", [CW, D], F32, kind="ExternalInput").ap()
    qkvb_d = nc.dram_tensor("qkvb", [3 * CW], F32, kind="ExternalInput").ap()
    y_d = nc.dram_tensor("y", [N, D], F32, kind="ExternalOutput").ap()

    with tile.TileContext(nc) as tc:
        for rep in range(reps):
            rsc_d = nc.dram_tensor(f"rscratch{rep}", [16, 512], F32).ap()
            _body(nc, tc, x_d, wq_d, wk_d, wv_d, wp_d, qkvb_d, y_d, rsc_d,
                  stages=stages)

    if split:
        _split_sync_waits(nc)
    return nc


def _body(nc, tc, x_d, wq_d, wk_d, wv_d, wp_d, qkvb_d, y_d, rsc_d, stages="ABC"):
    from contextlib import ExitStack

    persist = ExitStack()
    const_p = persist.enter_context(tc.tile_pool(name="const", bufs=1))
    qk_p = persist.enter_context(tc.tile_pool(name="qk", bufs=1))
    v1_p = persist.enter_context(tc.tile_pool(name="v1", bufs=1))

    # ---- persistent PSUM pools (8 banks total, fixed for whole kernel) ----
    st_p = persist.enter_context(tc.tile_pool(name="st", bufs=2, space="PSUM"))
    us_p = persist.enter_context(tc.tile_pool(name="us", bufs=1, space="PSUM"))
    aux_p = persist.enter_context(tc.tile_pool(name="aux", bufs=2, space="PSUM"))

    et_p = persist.enter_context(tc.tile_pool(name="et", bufs=ULAG + 1))

    ident = const_p.tile([128, 128], F32)
    make_identity(nc, ident)

    # Pre-warm the exp activation table set on ACT (2.7us table DMA) so the
    # first real exp doesn't pay it mid-pipeline.
    warm = const_p.tile([128, 1], F32)
    nc.scalar.activation(warm, ident[:, 0:1], AF.Exp)

    qT = qk_p.tile([128, 2, N], F32)      # [c-in-pair, pair, n]
    kT = qk_p.tile([128, 2, N], F32)
    v1 = v1_p.tile([128, NT, HL, HD + 1], F32)   # ones in last column

    qb = const_p.tile([128, 2], F32)
    kb = const_p.tile([128, 2], F32)
    vbc = const_p.tile([128, CW], F32)
    for pair in range(2):
        nc.gpsimd.dma_start(qb[:, pair: pair + 1],
                            qkvb_d[bass.ds(pair * 128, 128)].unsqueeze(1))
        nc.gpsimd.dma_start(kb[:, pair: pair + 1],
                            qkvb_d[bass.ds(CW + pair * 128, 128)].unsqueeze(1))
    nc.gpsimd.dma_start(
        vbc,
        qkvb_d[bass.ds(2 * CW, CW)].unsqueeze(0).partition_broadcast(128).squeeze(1))

    ones64 = const_p.tile([1, HD], F32)
    nc.vector.tensor_scalar(_r(ones64), vbc[0:1, 0:HD],
                            0.0, 1.0, ALU.mult, ALU.add)

    # ones column of v1 (DVE memset cannot emit f32r; use in0*0 + 1)
    nc.vector.tensor_scalar(
        _r(v1[:, :, :, HD]),
        vbc[:, 0:NT * HL].rearrange("p (a b) -> p a b", a=NT),
        0.0, 1.0, ALU.mult, ALU.add)

    # ---------------- Stage A pools (right side) --------------------------
    sa = ExitStack()    # w + xT: alive until the last residual qk matmul
    sa1 = ExitStack()   # x staging + wv + wraw: freed earlier
    w_p = sa.enter_context(tc.tile_pool(name="w", bufs=1, side="right"))
    xT_p = sa.enter_context(tc.tile_pool(name="xT", bufs=1, side="right"))
    wv_p = sa1.enter_context(tc.tile_pool(name="wv", bufs=1, side="right"))
    wraw_p = sa1.enter_context(tc.tile_pool(name="wraw", bufs=1, side="right"))
    xs_p = sa1.enter_context(tc.tile_pool(name="xs", bufs=9, side="right"))

    wq_s = w_p.tile([128, KC, CW], F32)
    wk_s = w_p.tile([128, KC, CW], F32)
    wv_s = wv_p.tile([128, KC, CW], F32)

    def load_weights():
        for (wd, ws) in ((wv_d, wv_s), (wq_d, wq_s), (wk_d, wk_s)):
            wr = wraw_p.tile([128, KC, CW], F32, tag="wraw", name="wraw")
            nc.gpsimd.dma_start(wr, wd.rearrange("(t p) c -> p t c", p=128))
            nc.vector.tensor_copy(_r(ws), wr)

    xT = xT_p.tile([128, KC, N], F32)

    def tg_load(g):
        """g indexes groups of 4 n-tiles (512 rows)."""
        xts = []
        for i in range(4):
            xt = xs_p.tile([128, D], F32, tag="xs", name="xs")
            nc.sync.dma_start(xt, x_d[bass.ds((g * 4 + i) * 128, 128), :])
            xts.append(xt)
        return xts

    def tg_dc(xts, g, dc):
        pt = aux_p.tile([128, 512], F32, tag="aux", name="aux")
        for i in range(4):
            nc.tensor.transpose(
                pt[:, i * 128:(i + 1) * 128],
                xts[i][:, dc * 128:(dc + 1) * 128],
                ident)
        nc.vector.tensor_copy(_r(xT[:, dc, bass.ds(g * 512, 512)]), pt)

    def emit_v(mt):
        ps = aux_p.tile([128, 512], F32, tag="aux", name="aux")
        for dc in range(KC):
            nc.tensor.matmul(
                ps[:, 0:CW],
                _mm(xT[:, dc, bass.ds(mt * 128, 128)]),
                _mm(wv_s[:, dc, :]),
                start=(dc == 0), stop=(dc == KC - 1))
        nc.vector.tensor_add(
            _r(v1[:, mt, :, 0:HD]),
            ps[:, 0:CW].rearrange("p (h d) -> p h d", h=HL),
            vbc.rearrange("p (h d) -> p h d", h=HL))

    def emit_qk(pair, which, nb4):
        """Whole q/k chunk at once (prefix use)."""
        steps = qk_chunk_steps(pair, which, nb4)
        for s in steps:
            s()

    def qk_chunk_steps(pair, which, nb4):
        """The same chunk cut into 8 one-matmul steps + fused drain."""
        wt, dst, bias = ((wq_s, qT, qb), (wk_s, kT, kb))[which]
        cell = {}

        def step(dc):
            def go():
                if dc == 0:
                    cell["ps"] = aux_p.tile([128, 512], F32, tag="aux",
                                            name="aux")
                nc.tensor.matmul(
                    cell["ps"],
                    _mm(wt[:, dc, bass.ds(pair * 128, 128)]),
                    _mm(xT[:, dc, bass.ds(nb4 * 512, 512)]),
                    start=(dc == 0), stop=(dc == KC - 1))
                if dc == KC - 1:
                    nc.vector.tensor_scalar(
                        _r(dst[:, pair, bass.ds(nb4 * 512, 512)]),
                        cell["ps"], bias[:, pair: pair + 1], None, ALU.add)
            return go
        return [step(dc) for dc in range(KC)]

    # --------------------------- prefix -----------------------------------
    xts01 = [tg_load(0), tg_load(1)]
    load_weights()
    for g in (0, 1):
        for dc in range(KC):
            tg_dc(xts01[g], g, dc)
    xts2 = tg_load(2)
    xts3 = tg_load(3)
    for g, xts in ((2, xts2), (3, xts3)):
        for dc in range(KC):
            tg_dc(xts, g, dc)
    for mt in range(NT):
        emit_v(mt)
    sa1.close()
    for nb4 in range(NCK):
        emit_qk(0, 1, nb4)        # kT pair0, all chunks
    emit_qk(0, 0, 0)              # qT pair0, chunk 0
    emit_qk(1, 0, 0)              # qT pair1, chunk 0 (block 3 dep)

    if "B" not in stages:
        sa.close()
        persist.close()
        return

    # ---------------- Stage B + C ------------------------------------------
    ot_p = persist.enter_context(tc.tile_pool(name="ot", bufs=1))
    OT = ot_p.tile([128, 2, N], F32)   # [c-in-pair, pair, n]
    ri_p = persist.enter_context(tc.tile_pool(name="ri", bufs=2))
    rb_p = persist.enter_context(tc.tile_pool(name="rb", bufs=2))
    otu_p = persist.enter_context(tc.tile_pool(name="otu", bufs=2))
    y_p = persist.enter_context(tc.tile_pool(name="y", bufs=2))
    wp_p = persist.enter_context(tc.tile_pool(name="wp", bufs=1))
    wp_s = wp_p.tile([128, 2, D], F32)
    wp_raw = wp_p.tile([128, 2, D], F32)
    nc.gpsimd.dma_start(wp_raw, wp_d.rearrange("(t p) e -> p t e", p=128))
    nc.vector.tensor_copy(_r(wp_s), wp_raw)

    do_c = "C" in stages

    def proj_steps(nck):
        """Projection for n-chunk nck: 4 nts x 2 ec halves, one step each."""
        cells = [{} for _ in range(4)]

        def step(i, ec):
            nt = nck * 4 + i

            def go():
                if ec == 0:
                    cells[i]["yt"] = y_p.tile([128, D], F32, tag="y",
                                              name="yt")
                ps = aux_p.tile([128, 512], F32, tag="aux", name="aux")
                for pair in range(2):
                    nc.tensor.matmul(
                        ps,
                        _mm(OT[:, pair, bass.ds(nt * 128, 128)]),
                        _mm(wp_s[:, pair, bass.ds(ec * 512, 512)]),
                        start=(pair == 0), stop=(pair == 1))
                yt = cells[i]["yt"]
                nc.vector.tensor_copy(yt[:, bass.ds(ec * 512, 512)], ps)
                if ec == 1:
                    nc.sync.dma_start(y_d[bass.ds(nt * 128, 128), :], yt)
            return go
        return [step(i, ec) for i in range(4) for ec in range(2)]

    # residual work: steps interleaved into stage-B mt slots, per block.
    # Every chunk lands >=1 block before the block whose STs consume it.
    resid = [[] for _ in range(8)]
    resid[0] = qk_chunk_steps(0, 0, 1) + qk_chunk_steps(1, 1, 0)
    resid[1] = qk_chunk_steps(0, 0, 2) + qk_chunk_steps(1, 1, 1)
    resid[2] = qk_chunk_steps(1, 1, 2) + qk_chunk_steps(1, 1, 3)
    resid[3] = qk_chunk_steps(1, 0, 1)
    resid[4] = qk_chunk_steps(1, 0, 2) + (proj_steps(0) if do_c else [])
    resid[5] = qk_chunk_steps(0, 0, 3) + (proj_steps(1) if do_c else [])
    resid[6] = qk_chunk_steps(1, 0, 3) + (proj_steps(2) if do_c else [])

    BLOCKS = [(0, 0), (1, 0), (2, 0), (0, 1), (1, 1), (2, 1), (3, 0), (3, 1)]

    def emit_block(bi, nck, pair):
        us = [us_p.tile([HD + 1, 512], F32, tag=f"u{sub}", name=f"u{sub}")
              for sub in range(2)]
        ets = {}
        steps = list(resid[bi])
        si = 0

        def do_st(mt):
            st = st_p.tile([128, 1024], F32, tag="st", name="st")
            for sub in range(2):
                nc.tensor.matmul(
                    st[:, sub * 512:(sub + 1) * 512],
                    _mm(kT[bass.ds(sub * HD, HD), pair,
                           bass.ds(mt * 128, 128)]),
                    _mm(qT[bass.ds(sub * HD, HD), pair,
                           bass.ds(nck * 512, 512)]),
                    start=True, stop=True,
                    tile_position=(sub * HD, 0))
            et = et_p.tile([128, 1024], F32, tag="et", name="et")
            nc.scalar.activation(_r(et), st, AF.Exp, scale=float(SCALE))
            ets[mt] = et

        def do_u(mt):
            et = ets.pop(mt)
            for sub in range(2):
                nc.tensor.matmul(
                    us[sub],
                    _mm(v1[:, mt, pair * 2 + sub, :]),
                    _mm(et[:, sub * 512:(sub + 1) * 512]),
                    start=(mt == 0), stop=(mt == NT - 1))

        for mt in range(NT):
            do_st(mt)
            if si < len(steps):
                steps[si]()
                si += 1
            if mt >= ULAG:
                do_u(mt - ULAG)
        for mt in range(NT - ULAG, NT):
            do_u(mt)
        while si < len(steps):
            steps[si]()
            si += 1

        # normalize: OT[:, pair, nck-chunk] = U / den.  For the final
        # block the DRAM-bounce broadcast sits on the critical path into
        # the tail projection; a K=1 PE matmul (ones64^T @ ri) broadcasts
        # the reciprocal into an idle aux PSUM bank instead.
        last = (bi == len(BLOCKS) - 1)
        for sub in range(2):
            idx = bi * 2 + sub
            otu = otu_p.tile([HD, 512], F32, tag="otu", name="otu")
            nc.vector.tensor_copy(otu, us[sub][0:HD, :])
            ri = ri_p.tile([1, 512], F32, tag="ri", name="ri")
            if last:
                with nc.allow_low_precision(reason="f32r view is fp32 bits"):
                    nc.vector.reciprocal(_r(ri), us[sub][HD:HD + 1, :])
                rbp = aux_p.tile([128, 512], F32, tag="aux", name="aux")
                nc.tensor.matmul(rbp[0:HD, :], _mm(ones64), _mm(ri),
                                 start=True, stop=True)
                rb_ap = rbp[0:HD, :]
            else:
                nc.vector.reciprocal(ri, us[sub][HD:HD + 1, :])
                nc.sync.dma_start(rsc_d[idx: idx + 1, :], ri)
                rb = rb_p.tile([HD, 512], F32, tag="rb", name="rb")
                nc.sync.dma_start(
                    rb,
                    rsc_d[idx, :].unsqueeze(0)
                    .partition_broadcast(HD).squeeze(1))
                rb_ap = rb
            nc.vector.tensor_mul(
                _r(OT[bass.ds(sub * HD, HD), pair,
                      bass.ds(nck * 512, 512)]),
                otu, rb_ap)

    for bi, (nck, pair) in enumerate(BLOCKS):
        emit_block(bi, nck, pair)

    sa.close()

    # tail: projection for the last n-chunk
    if do_c:
        for s in proj_steps(3):
            s()

    persist.close()


_NC_CACHE = None


def _get_program():
    global _NC_CACHE
    if _NC_CACHE is None:
        _NC_CACHE = _build_program()
    return _NC_CACHE


def make_in_maps(x, qkv_w, qkv_b, proj_w):
    in_maps = []
    for c in range(NC):
        b, j = divmod(c, NC // B)
        cs = j * CW
        in_maps.append({
            "x": np.ascontiguousarray(x[b], np.float32),
            "wq": np.ascontiguousarray(qkv_w[:, cs: cs + CW], np.float32),
            "wk": np.ascontiguousarray(qkv_w[:, D + cs: D + cs + CW], np.float32),
            "wv": np.ascontiguousarray(qkv_w[:, 2 * D + cs: 2 * D + cs + CW], np.float32),
            "wp": np.ascontiguousarray(proj_w[cs: cs + CW, :], np.float32),
            "qkvb": np.concatenate([
                qkv_b[cs: cs + CW],
                qkv_b[D + cs: D + cs + CW],
                qkv_b[2 * D + cs: 2 * D + cs + CW]]).astype(np.float32),
        })
    return in_maps


def combine_outputs(results, proj_b):
    out = np.empty((B, N, D), np.float32)
    per = NC // B
    for b in range(B):
        acc = results[b * per]["y"].astype(np.float32)
        for c in range(b * per + 1, (b + 1) * per):
            acc = acc + results[c]["y"]
        out[b] = acc + proj_b[None, :].astype(np.float32)
    return out


def kernel(**inputs):
    x = np.asarray(inputs["x"], np.float32)
    qkv_w = np.asarray(inputs["qkv_w"], np.float32)
    qkv_b = np.asarray(inputs["qkv_b"], np.float32)
    proj_w = np.asarray(inputs["proj_w"], np.float32)
    proj_b = np.asarray(inputs["proj_b"], np.float32)

    nc = _get_program()
    in_maps = make_in_maps(x, qkv_w, qkv_b, proj_w)
    res = run_bass_kernel_spmd(nc, in_maps, list(range(NC)), trace=False)
    return combine_outputs(res.results, proj_b)


# revision 21
# speedup vs baseline: 1.1515x; 1.1515x over previous
"""Multi-head attention (B=2, N=2048, D=1024, H=16) on 8 NeuronCores.

Sharding: data-parallel over batch (cores 0-3 -> b=0, cores 4-7 -> b=1),
tensor-parallel over heads (4 heads per core; column-parallel QKV,
row-parallel proj). Each core emits a partial projection output
y_c = O_heads(c) @ proj_w[rows(c)]; the host sums the 4 partials per batch
and adds proj_b.

v2 design (ACT-exp is the hard floor: 16.8M exps/core @ 1.2GHz ~= 110us):
  - The Scalar (ACT) engine does ONLY exp, one [128,1024] activate per
    (block, mt).  All copies / bias adds run on DVE.
  - Stage B runs as 8 blocks (n-chunk of 512 x head-pair).  Per mt:
    two K=64 score matmuls row-tiled to PE quadrants (0,0)/(64,0) so they
    execute concurrently, one exp, two U (E^T[V|1]) matmuls with M=65
    (row 64 accumulates the softmax denominator).
  - PSUM: st 2x[128,1024] (4 banks) + us 2x[65,512] (2 banks) +
    aux 2x[128,512] (2 banks) = 8 banks exactly.  aux serves transposes,
    QKV chunk accumulation, and proj accumulation.
  - Residual stage-A work (later q/k chunks) and the projection are cut
    into <=1024-cycle steps and statically interleaved into the PE slack
    of stage-B mt slots; U lags ST by ULAG slots so the block-boundary
    normalize (DVE) never stalls the exp cadence.
  - Normalize: DVE copy of U rows, DVE reciprocal of the denominator row,
    DRAM-bounce broadcast, DVE multiply into OT (c-major), as in v1.
"""

import numpy as np

import concourse.bass as bass
import concourse.tile as tile
from concourse import mybir
from concourse.bass_utils import run_bass_kernel_spmd
from concourse.masks import make_identity

# ---- problem constants (hardcoded per contract) ----
B = 2
N = 2048
D = 1024
H = 16
HD = 64          # head dim
SCALE = HD ** -0.5
NC = 8           # cores
HL = H // (NC // B)   # heads per core = 4
CW = HL * HD     # local qkv column width = 256

F32 = mybir.dt.float32
F32R = mybir.dt.float32r

NT = N // 128    # 16 m-tiles
KC = D // 128    # 8 contraction chunks for qkv matmuls
NCK = N // 512   # 4 n-chunks (stage-B block columns)
NPRE = 2         # block-0 ST+exp pairs pre-fired in the prefix
                 # (= st bufs, so the prefire never stalls PE)
ULAG = 2         # U matmuls lag ST by this many mt slots

ALU = mybir.AluOpType
AF = mybir.ActivationFunctionType


def _mm(ap):
    return ap.bitcast(F32R)


def _r(ap):
    return ap.bitcast(F32R)


def _split_sync_waits(nc, maxw: int = 1) -> int:
    """This walrus build rejects >1 semaphore-wait per instruction
    (setupSyncWait: "Too many sync wait commands"). Hoist excess waits
    onto preceding same-engine no-ops: the sequencer runs instructions
    in order, so the semantics are unchanged."""
    n_split = 0
    for fn in nc.m.functions:
        for bb in fn.blocks:
            insts = list(bb.instructions)
            out = []
            changed = False
            for inst in insts:
                si = inst.sync_info
                waits = list(si.on_wait) if si is not None and si.on_wait else []
                if len(waits) > maxw:
                    chunks = [waits[i: i + maxw] for i in range(0, len(waits), maxw)]
                    for chunk in chunks[:-1]:
                        out.append(mybir.InstNoOp(
                            name=f"I-splitw-{nc.next_id()}",
                            sync_info=mybir.SyncInfo(on_wait=chunk, on_update=[]),
                            bass_nofuse=True,
                            engine=inst.engine,
                        ))
                    si.on_wait = chunks[-1]
                    inst.sync_info = si
                    n_split += 1
                    changed = True
                out.append(inst)
            if changed:
                try:
                    bb.instructions = out
                except Exception:
                    bb.instructions.clear()
                    for i in out:
                        bb.instructions.append(i)
    return n_split


def _build_program(split=True, reps=1, stages="ABC"):
    nc = bass.Bass(trn_type="TRN2", target_bir_lowering=False, debug=False)

    x_d = nc.dram_tensor("x", [N, D], F32, kind="ExternalInput").ap()
    wq_d = nc.dram_tensor("wq", [D, CW], F32, kind="ExternalInput").ap()
    wk_d = nc.dram_tensor("wk", [D, CW], F32, kind="ExternalInput").ap()
    wv_d = nc.dram_tensor("wv", [D, CW], F32, kind="ExternalInput").ap()
    wp_d = nc.dram_tensor("wp", [CW, D], F32, kind="ExternalInput").ap()
    qkvb_d = nc.dram_tensor("qkvb", [3 * CW], F32, kind="ExternalInput").ap()
    y_d = nc.dram_tensor("y", [N, D], F32, kind="ExternalOutput").ap()

    with tile.TileContext(nc) as tc:
        for rep in range(reps):
            rsc_d = nc.dram_tensor(f"rscratch{rep}", [16, 512], F32).ap()
            _body(nc, tc, x_d, wq_d, wk_d, wv_d, wp_d, qkvb_d, y_d, rsc_d,
                  stages=stages)

    if split:
        _split_sync_waits(nc)
    return nc


def _body(nc, tc, x_d, wq_d, wk_d, wv_d, wp_d, qkvb_d, y_d, rsc_d, stages="ABC"):
    from contextlib import ExitStack

    persist = ExitStack()
    const_p = persist.enter_context(tc.tile_pool(name="const", bufs=1))
    qk_p = persist.enter_context(tc.tile_pool(name="qk", bufs=1))
    v1_p = persist.enter_context(tc.tile_pool(name="v1", bufs=1))

    # ---- persistent PSUM pools (8 banks total, fixed for whole kernel) ----
    st_p = persist.enter_context(tc.tile_pool(name="st", bufs=2, space="PSUM"))
    us_p = persist.enter_context(tc.tile_pool(name="us", bufs=1, space="PSUM"))
    aux_p = persist.enter_context(tc.tile_pool(name="aux", bufs=2, space="PSUM"))

    et_p = persist.enter_context(tc.tile_pool(name="et", bufs=ULAG + 1))

    ident = const_p.tile([128, 128], F32)
    make_identity(nc, ident)

    # Pre-warm the exp activation table set on ACT (2.7us table DMA) so the
    # first real exp doesn't pay it mid-pipeline.
    warm = const_p.tile([128, 1], F32)
    nc.scalar.activation(warm, ident[:, 0:1], AF.Exp)

    qT = qk_p.tile([128, 2, N], F32)      # [c-in-pair, pair, n]
    kT = qk_p.tile([128, 2, N], F32)
    v1 = v1_p.tile([128, NT, HL, HD + 1], F32)   # ones in last column

    qb = const_p.tile([128, 2], F32)
    kb = const_p.tile([128, 2], F32)
    vbc = const_p.tile([128, CW], F32)
    for pair in range(2):
        nc.gpsimd.dma_start(qb[:, pair: pair + 1],
                            qkvb_d[bass.ds(pair * 128, 128)].unsqueeze(1))
        nc.gpsimd.dma_start(kb[:, pair: pair + 1],
                            qkvb_d[bass.ds(CW + pair * 128, 128)].unsqueeze(1))
    nc.gpsimd.dma_start(
        vbc,
        qkvb_d[bass.ds(2 * CW, CW)].unsqueeze(0).partition_broadcast(128).squeeze(1))

    ones64 = const_p.tile([1, HD], F32)
    nc.vector.tensor_scalar(_r(ones64), vbc[0:1, 0:HD],
                            0.0, 1.0, ALU.mult, ALU.add)

    # ones column of v1 (DVE memset cannot emit f32r; use in0*0 + 1)
    nc.vector.tensor_scalar(
        _r(v1[:, :, :, HD]),
        vbc[:, 0:NT * HL].rearrange("p (a b) -> p a b", a=NT),
        0.0, 1.0, ALU.mult, ALU.add)

    # ---------------- Stage A pools (right side) --------------------------
    sa = ExitStack()    # w + xT: alive until the last residual qk matmul
    sa1 = ExitStack()   # x staging + wv + wraw: freed earlier
    w_p = sa.enter_context(tc.tile_pool(name="w", bufs=1, side="right"))
    xT_p = sa.enter_context(tc.tile_pool(name="xT", bufs=1, side="right"))
    wv_p = sa1.enter_context(tc.tile_pool(name="wv", bufs=1, side="right"))
    wraw_p = sa1.enter_context(tc.tile_pool(name="wraw", bufs=1, side="right"))
    xs_p = sa1.enter_context(tc.tile_pool(name="xs", bufs=9, side="right"))

    wq_s = w_p.tile([128, KC, CW], F32)
    wk_s = w_p.tile([128, KC, CW], F32)
    wv_s = wv_p.tile([128, KC, CW], F32)

    def load_weights():
        # wk first: the early kp0c0 chunk (prefire dependency) needs it
        for (wd, ws) in ((wk_d, wk_s), (wq_d, wq_s), (wv_d, wv_s)):
            wr = wraw_p.tile([128, KC, CW], F32, tag="wraw", name="wraw")
            nc.gpsimd.dma_start(wr, wd.rearrange("(t p) c -> p t c", p=128))
            nc.vector.tensor_copy(_r(ws), wr)

    xT = xT_p.tile([128, KC, N], F32)

    def tg_load(g):
        """g indexes groups of 4 n-tiles (512 rows)."""
        xts = []
        for i in range(4):
            xt = xs_p.tile([128, D], F32, tag="xs", name="xs")
            nc.sync.dma_start(xt, x_d[bass.ds((g * 4 + i) * 128, 128), :])
            xts.append(xt)
        return xts

    def tg_dc(xts, g, dc):
        pt = aux_p.tile([128, 512], F32, tag="aux", name="aux")
        for i in range(4):
            nc.tensor.transpose(
                pt[:, i * 128:(i + 1) * 128],
                xts[i][:, dc * 128:(dc + 1) * 128],
                ident)
        nc.vector.tensor_copy(_r(xT[:, dc, bass.ds(g * 512, 512)]), pt)

    def emit_v(mt):
        ps = aux_p.tile([128, 512], F32, tag="aux", name="aux")
        for dc in range(KC):
            nc.tensor.matmul(
                ps[:, 0:CW],
                _mm(xT[:, dc, bass.ds(mt * 128, 128)]),
                _mm(wv_s[:, dc, :]),
                start=(dc == 0), stop=(dc == KC - 1))
        nc.vector.tensor_add(
            _r(v1[:, mt, :, 0:HD]),
            ps[:, 0:CW].rearrange("p (h d) -> p h d", h=HL),
            vbc.rearrange("p (h d) -> p h d", h=HL))

    def emit_qk(pair, which, nb4):
        """Whole q/k chunk at once (prefix use)."""
        steps = qk_chunk_steps(pair, which, nb4)
        for s in steps:
            s()

    def qk_chunk_steps(pair, which, nb4):
        """The same chunk cut into 8 one-matmul steps + fused drain."""
        wt, dst, bias = ((wq_s, qT, qb), (wk_s, kT, kb))[which]
        cell = {}

        def step(dc):
            def go():
                if dc == 0:
                    cell["ps"] = aux_p.tile([128, 512], F32, tag="aux",
                                            name="aux")
                nc.tensor.matmul(
                    cell["ps"],
                    _mm(wt[:, dc, bass.ds(pair * 128, 128)]),
                    _mm(xT[:, dc, bass.ds(nb4 * 512, 512)]),
                    start=(dc == 0), stop=(dc == KC - 1))
                if dc == KC - 1:
                    nc.vector.tensor_scalar(
                        _r(dst[:, pair, bass.ds(nb4 * 512, 512)]),
                        cell["ps"], bias[:, pair: pair + 1], None, ALU.add)
            return go
        return [step(dc) for dc in range(KC)]

    # --------------------------- prefix -----------------------------------
    # Block 0's first score tiles depend only on group-0 transposes plus
    # k/q pair-0 chunk 0; fire those STs + exps here so the ACT cadence
    # starts ~3us before the prefix ends (NPRE limited by the et pool).
    pre_ets = []

    def prefire(nck, pair):
        for mt in range(NPRE):
            st = st_p.tile([128, 1024], F32, tag="st", name="st")
            for sub in range(2):
                nc.tensor.matmul(
                    st[:, sub * 512:(sub + 1) * 512],
                    _mm(kT[bass.ds(sub * HD, HD), pair,
                           bass.ds(mt * 128, 128)]),
                    _mm(qT[bass.ds(sub * HD, HD), pair,
                           bass.ds(nck * 512, 512)]),
                    start=True, stop=True,
                    tile_position=(sub * HD, 0))
            et = et_p.tile([128, 1024], F32, tag="et", name="et")
            nc.scalar.activation(_r(et), st, AF.Exp, scale=float(SCALE))
            pre_ets.append(et)

    xts01 = [tg_load(0), tg_load(1)]
    load_weights()
    for g in (0, 1):
        for dc in range(KC):
            tg_dc(xts01[g], g, dc)
    emit_qk(0, 1, 0)              # kT pair0, chunk 0
    emit_qk(0, 0, 0)              # qT pair0, chunk 0
    prefire(0, 0)                 # block 0 = (nck 0, pair 0)
    xts2 = tg_load(2)
    xts3 = tg_load(3)
    for g, xts in ((2, xts2), (3, xts3)):
        for dc in range(KC):
            tg_dc(xts, g, dc)
    for mt in range(NT):
        emit_v(mt)
    sa1.close()
    for nb4 in range(1, NCK):
        emit_qk(0, 1, nb4)        # kT pair0, chunks 1-3
    emit_qk(1, 0, 0)              # qT pair1, chunk 0 (block 3 dep)

    if "B" not in stages:
        sa.close()
        persist.close()
        return

    # ---------------- Stage B + C ------------------------------------------
    ot_p = persist.enter_context(tc.tile_pool(name="ot", bufs=1))
    OT = ot_p.tile([128, 2, N], F32)   # [c-in-pair, pair, n]
    ri_p = persist.enter_context(tc.tile_pool(name="ri", bufs=2))
    rb_p = persist.enter_context(tc.tile_pool(name="rb", bufs=2))
    otu_p = persist.enter_context(tc.tile_pool(name="otu", bufs=2))
    y_p = persist.enter_context(tc.tile_pool(name="y", bufs=2))
    wp_p = persist.enter_context(tc.tile_pool(name="wp", bufs=1))
    wp_s = wp_p.tile([128, 2, D], F32)
    wp_raw = wp_p.tile([128, 2, D], F32)
    nc.gpsimd.dma_start(wp_raw, wp_d.rearrange("(t p) e -> p t e", p=128))
    nc.vector.tensor_copy(_r(wp_s), wp_raw)

    do_c = "C" in stages

    def proj_steps(nck):
        """Projection for n-chunk nck: 4 nts x 2 ec halves, one step each."""
        cells = [{} for _ in range(4)]

        def step(i, ec):
            nt = nck * 4 + i

            def go():
                if ec == 0:
                    cells[i]["yt"] = y_p.tile([128, D], F32, tag="y",
                                              name="yt")
                ps = aux_p.tile([128, 512], F32, tag="aux", name="aux")
                for pair in range(2):
                    nc.tensor.matmul(
                        ps,
                        _mm(OT[:, pair, bass.ds(nt * 128, 128)]),
                        _mm(wp_s[:, pair, bass.ds(ec * 512, 512)]),
                        start=(pair == 0), stop=(pair == 1))
                yt = cells[i]["yt"]
                nc.vector.tensor_copy(yt[:, bass.ds(ec * 512, 512)], ps)
                if ec == 1:
                    nc.sync.dma_start(y_d[bass.ds(nt * 128, 128), :], yt)
            return go
        return [step(i, ec) for i in range(4) for ec in range(2)]

    # residual work: steps interleaved into stage-B mt slots, per block.
    # Every chunk lands >=1 block before the block whose STs consume it.
    resid = [[] for _ in range(8)]
    resid[0] = qk_chunk_steps(0, 0, 1) + qk_chunk_steps(1, 1, 0)
    resid[1] = qk_chunk_steps(0, 0, 2) + qk_chunk_steps(1, 1, 1)
    resid[2] = qk_chunk_steps(1, 1, 2) + qk_chunk_steps(1, 1, 3)
    resid[3] = qk_chunk_steps(1, 0, 1)
    resid[4] = qk_chunk_steps(1, 0, 2) + (proj_steps(0) if do_c else [])
    resid[5] = qk_chunk_steps(0, 0, 3) + (proj_steps(1) if do_c else [])
    resid[6] = qk_chunk_steps(1, 0, 3) + (proj_steps(2) if do_c else [])

    BLOCKS = [(0, 0), (1, 0), (2, 0), (0, 1), (1, 1), (2, 1), (3, 0), (3, 1)]

    def emit_block(bi, nck, pair):
        us = [us_p.tile([HD + 1, 512], F32, tag=f"u{sub}", name=f"u{sub}")
              for sub in range(2)]
        ets = {mt: et for mt, et in enumerate(pre_ets)} if bi == 0 else {}
        npre = len(ets)
        steps = list(resid[bi])
        si = 0

        def do_st(mt):
            st = st_p.tile([128, 1024], F32, tag="st", name="st")
            for sub in range(2):
                nc.tensor.matmul(
                    st[:, sub * 512:(sub + 1) * 512],
                    _mm(kT[bass.ds(sub * HD, HD), pair,
                           bass.ds(mt * 128, 128)]),
                    _mm(qT[bass.ds(sub * HD, HD), pair,
                           bass.ds(nck * 512, 512)]),
                    start=True, stop=True,
                    tile_position=(sub * HD, 0))
            et = et_p.tile([128, 1024], F32, tag="et", name="et")
            nc.scalar.activation(_r(et), st, AF.Exp, scale=float(SCALE))
            ets[mt] = et

        def do_u(mt):
            et = ets.pop(mt)
            for sub in range(2):
                nc.tensor.matmul(
                    us[sub],
                    _mm(v1[:, mt, pair * 2 + sub, :]),
                    _mm(et[:, sub * 512:(sub + 1) * 512]),
                    start=(mt == 0), stop=(mt == NT - 1))

        for mt in range(NT):
            if mt >= npre:
                do_st(mt)
            if si < len(steps):
                steps[si]()
                si += 1
            if mt >= ULAG:
                do_u(mt - ULAG)
        for mt in range(NT - ULAG, NT):
            do_u(mt)
        while si < len(steps):
            steps[si]()
            si += 1

        # normalize: OT[:, pair, nck-chunk] = U / den.  For the final
        # block the DRAM-bounce broadcast sits on the critical path into
        # the tail projection; a K=1 PE matmul (ones64^T @ ri) broadcasts
        # the reciprocal into an idle aux PSUM bank instead.
        last = (bi == len(BLOCKS) - 1)
        for sub in range(2):
            idx = bi * 2 + sub
            otu = otu_p.tile([HD, 512], F32, tag="otu", name="otu")
            nc.vector.tensor_copy(otu, us[sub][0:HD, :])
            ri = ri_p.tile([1, 512], F32, tag="ri", name="ri")
            if last:
                with nc.allow_low_precision(reason="f32r view is fp32 bits"):
                    nc.vector.reciprocal(_r(ri), us[sub][HD:HD + 1, :])
                rbp = aux_p.tile([128, 512], F32, tag="aux", name="aux")
                nc.tensor.matmul(rbp[0:HD, :], _mm(ones64), _mm(ri),
                                 start=True, stop=True)
                rb_ap = rbp[0:HD, :]
            else:
                nc.vector.reciprocal(ri, us[sub][HD:HD + 1, :])
                nc.sync.dma_start(rsc_d[idx: idx + 1, :], ri)
                rb = rb_p.tile([HD, 512], F32, tag="rb", name="rb")
                nc.sync.dma_start(
                    rb,
                    rsc_d[idx, :].unsqueeze(0)
                    .partition_broadcast(HD).squeeze(1))
                rb_ap = rb
            nc.vector.tensor_mul(
                _r(OT[bass.ds(sub * HD, HD), pair,
                      bass.ds(nck * 512, 512)]),
                otu, rb_ap)

    for bi, (nck, pair) in enumerate(BLOCKS):
        emit_block(bi, nck, pair)

    sa.close()

    # tail: projection for the last n-chunk
    if do_c:
        for s in proj_steps(3):
            s()

    persist.close()


_NC_CACHE = None


def _get_program():
    global _NC_CACHE
    if _NC_CACHE is None:
        _NC_CACHE = _build_program()
    return _NC_CACHE


def make_in_maps(x, qkv_w, qkv_b, proj_w):
    in_maps = []
    for c in range(NC):
        b, j = divmod(c, NC // B)
        cs = j * CW
        in_maps.append({
            "x": np.ascontiguousarray(x[b], np.float32),
            "wq": np.ascontiguousarray(qkv_w[:, cs: cs + CW], np.float32),
            "wk": np.ascontiguousarray(qkv_w[:, D + cs: D + cs + CW], np.float32),
            "wv": np.ascontiguousarray(qkv_w[:, 2 * D + cs: 2 * D + cs + CW], np.float32),
            "wp": np.ascontiguousarray(proj_w[cs: cs + CW, :], np.float32),
            "qkvb": np.concatenate([
                qkv_b[cs: cs + CW],
                qkv_b[D + cs: D + cs + CW],
                qkv_b[2 * D + cs: 2 * D + cs + CW]]).astype(np.float32),
        })
    return in_maps


def combine_outputs(results, proj_b):
    out = np.empty((B, N, D), np.float32)
    per = NC // B
    for b in range(B):
        acc = results[b * per]["y"].astype(np.float32)
        for c in range(b * per + 1, (b + 1) * per):
            acc = acc + results[c]["y"]
        out[b] = acc + proj_b[None, :].astype(np.float32)
    return out


def kernel(**inputs):
    x = np.asarray(inputs["x"], np.float32)
    qkv_w = np.asarray(inputs["qkv_w"], np.float32)
    qkv_b = np.asarray(inputs["qkv_b"], np.float32)
    proj_w = np.asarray(inputs["proj_w"], np.float32)
    proj_b = np.asarray(inputs["proj_b"], np.float32)

    nc = _get_program()
    in_maps = make_in_maps(x, qkv_w, qkv_b, proj_w)
    res = run_bass_kernel_spmd(nc, in_maps, list(range(NC)), trace=False)
    return combine_outputs(res.results, proj_b)
